# revision 1
# baseline (speedup 1.0000x reference)
"""BotRGCN Trainium2 kernel v2, 8-way SPMD.

Key changes vs v1:
- Dead-node pruning via `idx`: L2 computes only ~9k dst nodes, L1 only the
  ~45k nodes L2 actually reads (sigma permutation packs them tile-aligned).
- Self-loop/root contributions come from a constant one-hot matmul on the
  local contiguous h tile (no gather entries for them).
- A (scatter one-hot) matrices precomputed on host, streamed via HWDGE.
- W=128 windows; slot entries sorted by source row for HBM locality.
- Warmup collective at kernel start so the real AllGathers run warm.
"""
import os
import sys

for _p in ("/opt/trn_rl_repo", "/root/.axon_site/_ro/trn_rl_repo"):
    if os.path.isdir(_p) and _p not in sys.path:
        sys.path.insert(0, _p)

import numpy as np
import ml_dtypes

from concourse import bass, bacc, tile, mybir
from concourse.bass_utils import run_bass_kernel_spmd

BF16 = ml_dtypes.bfloat16

N_NODES = 50000
N_REL = 3
FEAT = 128
VAL = 16
TEXT = 768
CLASSES = 2
CORES = 8
P = 128
W = 128
CHMAX = 4
ABATCH = 32
RSLOT = 4
NSP = ((N_NODES // CORES) + P - 1) // P * P  # 6272
NT_MLP = NSP // P                            # 49
TC = TEXT // P                               # 6



def wrap16(flat):
    L = len(flat)
    assert L % 16 == 0
    a = np.asarray(flat, np.int16).reshape(-1, 16).T
    return np.ascontiguousarray(np.tile(a, (8, 1)))


# ============================ planner ================================


class Plan:
    pass


def _build_schedule(cts, cmax, w=W, cap=P):
    """Joint (cross-core) slot schedule for one (tile, section).
    cts: list of sorted int arrays (ct keys in [0, cmax)).
    Returns (bases, ranges)."""
    n = len(cts)
    ptrs = [0] * n
    lens = [len(a) for a in cts]
    bases, ranges = [], [[] for _ in range(n)]
    while any(ptrs[c] < lens[c] for c in range(n)):
        b = min(cts[c][ptrs[c]] for c in range(n) if ptrs[c] < lens[c])
        b = min(int(b), cmax - w)
        bases.append(b)
        for c in range(n):
            s = ptrs[c]
            hi = int(np.searchsorted(cts[c], b + w, side="left"))
            e = min(s + cap, max(hi, s))
            ranges[c].append((s, e))
            ptrs[c] = e
    return bases, ranges


def make_sigma(edge_index, edge_type, idx, n_nodes, cores, nsp):
    """Node permutation + pruning sets.

    Returns dict with:
      pos: [n_nodes] -> slot position (c*nsp + l), -1 unused? (all nodes placed)
      shard_nodes: [cores, nsp] node id per slot (-1 pad)
      nt2, nt1: L2/L1 tiles per core
    """
    src = np.asarray(edge_index[0], np.int64)
    dst = np.asarray(edge_index[1], np.int64)

    idxset = np.unique(np.asarray(idx, np.int64))
    in_idx = np.zeros(n_nodes, bool)
    in_idx[idxset] = True

    m2 = in_idx[dst]                      # L2 edges
    l2src = np.unique(src[m2])
    needed = np.zeros(n_nodes, bool)
    needed[l2src] = True
    needed[idxset] = True
    m1 = needed[dst]                      # L1 edges

    D = idxset                             # idx dsts
    O = np.setdiff1d(np.flatnonzero(needed), D, assume_unique=False)
    U = np.flatnonzero(~needed)

    shard_lists = [[] for _ in range(cores)]
    for arr in (D, O, U):
        # round-robin but keep each class contiguous per shard
        pass
    # round-robin assignment preserving class order within each shard
    sd = [D[c::cores] for c in range(cores)]
    so = [O[c::cores] for c in range(cores)]
    su = [U[c::cores] for c in range(cores)]
    nd = max(len(x) for x in sd)
    nneed = max(len(sd[c]) + len(so[c]) for c in range(cores))
    nt2 = (nd + P - 1) // P
    nt1 = (nneed + P - 1) // P
    shard_nodes = np.full((cores, nsp), -1, np.int64)
    pos = np.full(n_nodes, -1, np.int64)
    for c in range(cores):
        arr = np.concatenate([sd[c], so[c], su[c]])
        assert len(arr) <= nsp, (len(arr), nsp)
        shard_nodes[c, :len(arr)] = arr
        pos[arr] = c * nsp + np.arange(len(arr))
    return dict(pos=pos, shard_nodes=shard_nodes, nt1=nt1, nt2=nt2,
                m1=m1, m2=m2, in_idx=in_idx, needed=needed,
                nd_per_core=np.array([len(x) for x in sd]))


def layer_plan(erow, ect, eowner, etile, enorm, cores, nt, sec,
               nsec=3):
    """Build joint slot schedule for one layer.

    erow: table-relative row per entry (int16 range); ect: ct key;
    eowner: dst core; etile: dst tile; enorm: edge norm; sec: table
    selector per entry in [0, nsec).
    """
    SECS = tuple(range(nsec))
    order = np.lexsort((ect, sec, etile, eowner))
    erow, ect, sec = erow[order], ect[order], sec[order]
    eowner, etile, enorm = eowner[order], etile[order], enorm[order]

    key = (eowner * nt + etile) * nsec + sec
    bounds = np.searchsorted(key, np.arange(cores * nt * nsec + 1))

    slot_base = {s: [] for s in SECS}
    tile_slot_range = {s: np.zeros((nt, 2), np.int64) for s in SECS}
    idx16 = {s: [[] for _ in range(cores)] for s in SECS}
    acols = {s: [[] for _ in range(cores)] for s in SECS}
    anrm = {s: [[] for _ in range(cores)] for s in SECS}

    for t in range(nt):
        for s in SECS:
            cts_, rows_, nrms_ = [], [], []
            for c in range(cores):
                k = (c * nt + t) * nsec + s
                a, b = bounds[k], bounds[k + 1]
                cts_.append(ect[a:b])
                rows_.append(erow[a:b])
                nrms_.append(enorm[a:b])
            start = len(slot_base[s])
            bases, ranges = _build_schedule(cts_, P * RSLOT)
            for bj in bases:
                slot_base[s].append(bj)
            for c in range(cores):
                for j, (a, b) in enumerate(ranges[c]):
                    n = b - a
                    r = rows_[c][a:b]
                    cc = cts_[c][a:b] - bases[j]
                    nn = nrms_[c][a:b]
                    # sort within slot by source row for HBM locality
                    so_ = np.argsort(r, kind="stable")
                    r, cc, nn = r[so_], cc[so_], nn[so_]
                    ii = np.zeros(P, np.int16)
                    col = np.full(P, -1, np.int64)
                    nrm = np.zeros(P, np.float32)
                    assert n == 0 or r.max() < 32768
                    ii[:n] = r.astype(np.int16)
                    col[:n] = cc
                    nrm[:n] = nn
                    idx16[s][c].append(ii)
                    acols[s][c].append(col)
                    anrm[s][c].append(nrm)
            tile_slot_range[s][t] = (start, len(slot_base[s]))

    ns = [len(slot_base[s]) for s in SECS]
    nslot = sum(ns)
    out = Plan()
    out.nt, out.nslot, out.ns, out.nsec = nt, nslot, ns, nsec
    out.slot_base = {s: np.array(slot_base[s], np.int64) for s in SECS}
    out.tile_slot_range = tile_slot_range

    # per-core packed: idx streams per sec; A matrix [128, nslot*W]
    out.idx = {}
    out.amat = {}
    for c in range(cores):
        out.idx[c] = [
            (np.stack(idx16[s][c]) if idx16[s][c] else np.zeros((0, P), np.int16))
            for s in SECS
        ]
        am = np.zeros((P, max(nslot, 1) * W), BF16)
        jg = 0
        for s in SECS:
            for j in range(len(idx16[s][c])):
                col = acols[s][c][j]
                nrm = anrm[s][c][j]
                v = col >= 0
                am[np.flatnonzero(v), jg * W + col[v]] = nrm[v].astype(BF16)
                jg += 1
        out.amat[c] = am
    return out


def make_plan(edge_index, edge_type, idx, n_nodes=50000, cores=8):
    src = np.asarray(edge_index[0], np.int64)
    dst = np.asarray(edge_index[1], np.int64)
    et = np.asarray(edge_type, np.int64)

    nsp = ((n_nodes // cores) + P - 1) // P * P  # 6272
    sg = make_sigma(edge_index, edge_type, idx, n_nodes, cores, nsp)
    pos, nt1, nt2 = sg["pos"], sg["nt1"], sg["nt2"]

    deg = np.zeros((N_REL, n_nodes), np.int64)
    np.add.at(deg, (et, dst), 1)
    norm = 1.0 / np.maximum(deg[et, dst], 1).astype(np.float32)

    pl = Plan()
    pl.cores, pl.nsp = cores, nsp
    pl.nt_mlp = nsp // P
    pl.sigma = sg
    pl.nt1, pl.nt2 = nt1, nt2

    # ---- L1: edges with dst in needed; h split across two tables:
    # table A = first splitA1 rows of each shard, table B = the rest.
    ntm = nsp // P                       # MLP tiles (49)
    splitA1 = ((ntm + 1) // 2) * P       # 25 tiles = 3200 rows
    splitB1 = nsp - splitA1              # 3072
    pl.split1 = (splitA1, splitB1)
    m1 = sg["m1"]
    s1, d1, r1, n1 = src[m1], dst[m1], et[m1], norm[m1]
    dpos = pos[d1]
    owner = dpos // nsp
    loc = dpos % nsp
    spos = pos[s1]
    sc, sl = spos // nsp, spos % nsp
    sec = (sl >= splitA1).astype(np.int64)
    erow = np.where(sec == 0, sc * splitA1 + sl,
                    sc * splitB1 + (sl - splitA1))
    assert erow.max() < 32768
    ect = (loc % P) * RSLOT + r1
    etile = loc // P
    assert (etile < nt1).all()
    pl.L1 = layer_plan(erow, ect, owner, etile, n1, cores, nt1, sec, nsec=2)

    # ---- L2: edges with dst in idxset; single h1 table, lo/hi sections
    m2 = sg["m2"]
    s2, d2, r2, n2 = src[m2], dst[m2], et[m2], norm[m2]
    dpos = pos[d2]
    owner = dpos // nsp
    loc = dpos % nsp
    assert (loc < nt2 * P).all()
    spos = pos[s2]
    sc, sl = spos // nsp, spos % nsp
    assert (sl < nt1 * P).all()
    h1row = sc * (nt1 * P) + sl
    nrows2 = cores * nt1 * P
    hibase2 = max(0, nrows2 - 32768)
    sec = (h1row >= 32768).astype(np.int64)
    erow = np.where(sec == 0, h1row, h1row - hibase2)
    assert erow.max() < 32768
    ect = (loc % P) * RSLOT + r2
    etile = loc // P
    pl.L2 = layer_plan(erow, ect, owner, etile, n2, cores, nt2, sec, nsec=2)
    pl.hibase2 = hibase2
    pl.nrows2 = nrows2
    return pl


# ---------------- numpy emulation for plan validation ----------------

def emulate_layer(pl, lp, h_tabs_per_core, w_rel, w_root, b, cores):
    """h_tabs_per_core[c]: tuple of tables per section, fp32."""
    F = h_tabs_per_core[0][0].shape[1]
    nt = lp.nt
    outs = []
    for c in range(cores):
        ST = np.zeros((nt, F, P * RSLOT), np.float32)  # per tile S^T
        h_tabs = h_tabs_per_core[c]
        secoff = np.cumsum([0] + lp.ns).tolist()
        for s in range(lp.nsec):
            idxs = lp.idx[c][s]
            for t in range(nt):
                a, b_ = lp.tile_slot_range[s][t]
                for j in range(a, b_):
                    jg_slot = j + secoff[s]
                    rows = idxs[j].astype(np.int64)
                    E = h_tabs[s][rows]  # [128, F]
                    A = lp.amat[c][:, jg_slot * W:(jg_slot + 1) * W].astype(np.float32)
                    bj = int(lp.slot_base[s][j])
                    ST[t, :, bj:bj + W] += E.T @ A
        out = np.zeros((nt * P, F), np.float32)
        for t in range(nt):
            S = ST[t]  # [F, 512]
            acc = np.zeros((P, F), np.float32)
            for r in range(N_REL):
                acc += S[:, r::RSLOT].T @ w_rel[r]
            # root handled by caller adding selfrow
            out[t * P:(t + 1) * P] = acc
        outs.append(out)
    return outs


# ============================ blob layout =============================

def blob_layout(pl):
    n1, n2 = pl.L1.nslot, pl.L2.nslot
    segs = [
        ("textT", [NT_MLP, P, TC * P]),
        ("valT", [VAL, NSP]),
        ("fc1w", [VAL, FEAT]),
        ("fc2w", [P, TC * P]),
        ("rwv", [FEAT, FEAT]),
        ("rwt", [FEAT, FEAT]),
        ("beff", [1, FEAT]),
        ("ww1", [P, RSLOT * FEAT]),
        ("b1", [1, FEAT]),
        ("ww2", [P, RSLOT * FEAT]),
        ("b2", [1, FEAT]),
        ("fc3w", [FEAT, CLASSES]),
        ("fc3b", [1, CLASSES]),
        ("ones1", [1, P]),
        ("selfA", [P, RSLOT * P]),
        ("a1", [P, max(n1, 1) * W]),
        ("a2", [P, max(n2, 1) * W]),
        ("idx1s0", [P, max(pl.L1.ns[0], 1) * 8]),
        ("idx1s1", [P, max(pl.L1.ns[1], 1) * 8]),
        ("idx2s0", [P, max(pl.L2.ns[0], 1) * 8]),
        ("idx2s1", [P, max(pl.L2.ns[1], 1) * 8]),
    ]
    out = {}
    off = 0
    for name, shape in segs:
        n = int(np.prod(shape))
        out[name] = (off, n, shape)
        off += ((n + 127) // 128) * 128
    return out, off


# ============================ bass builder =============================

def build_bass(pl, ablate=()):
    ab = set(ablate)
    NT1, NT2 = pl.nt1, pl.nt2
    NROWS1 = CORES * NSP
    NROWS2 = CORES * NT1 * P

    nc = bacc.Bacc("TRN2", target_bir_lowering=False, debug=False,
                   num_devices=CORES, num_swdge_queues=4,
                   dynamic_dma_scratch_size=49152)
    qrr = {"n": 0}
    dt = mybir.dt
    f32, bf, i16 = dt.float32, dt.bfloat16, dt.int16

    layout, blob_n = blob_layout(pl)
    p_blob = nc.declare_dram_parameter("blob", [1, blob_n], bf, isOutput=False)
    p_logT = nc.declare_dram_parameter("logitsT", [CLASSES, NT2 * P], f32,
                                       isOutput=True)

    def seg(name, dtype=bf):
        off, n, shape = layout[name]
        ap = p_blob[0:1, off:off + n]
        if dtype != bf:
            ap = ap.bitcast(dtype)
        r = int(np.prod(shape[:-1]))
        return ap.rearrange("o (r c) -> (o r) c", r=r)

    with tile.TileContext(nc) as tc:
        with tc.tile_pool(name="wt", bufs=1) as wt, \
             tc.tile_pool(name="sb", bufs=2) as sb, \
             tc.tile_pool(name="elo", bufs=16) as elo, \
             tc.tile_pool(name="ehi", bufs=16) as ehi, \
             tc.tile_pool(name="ab1", bufs=2) as abp, \
             tc.tile_pool(name="tts", bufs=3) as tts, \
             tc.tile_pool(name="dram", bufs=1, space="DRAM") as dram:

            def resident(name, dtype=bf):
                off, n, shape = layout[name]
                t = wt.tile(list(shape[-2:] if len(shape) == 2 else shape),
                            dtype, tag=name)
                nc.sync.dma_start(t[:], seg(name, dtype))
                return t

            fc1w = resident("fc1w")
            valT = resident("valT")
            fc2w = resident("fc2w")
            rwv = resident("rwv")
            rwt = resident("rwt")
            beff = resident("beff")
            ones1 = resident("ones1")

            hall1 = wt.tile([P, NT1, P], bf, tag="hall1")
            hall2 = wt.tile([P, NT2, P], bf, tag="hall2")

            sa1, sb1 = pl.split1
            h_shard = dram.tile([NSP, FEAT], bf)
            _as = "Shared" if "coll" not in ab else "Local"
            h_fullA = dram.tile([CORES * sa1, FEAT], bf, addr_space=_as)
            h_fullB = dram.tile([CORES * sb1, FEAT], bf, addr_space=_as)
            h1_shard = dram.tile([NT1 * P, FEAT], bf)
            h1_full = dram.tile([NROWS2, FEAT], bf, addr_space=_as)
            warm_in = dram.tile([P, 16], bf)
            warm_out = dram.tile([CORES * P, 16], bf, addr_space=_as)

            # -------- warmup collective (also the cross-core barrier) -----
            if "coll" not in ab:
                wz = sb.tile([P, 16], bf, tag="wz")
                nc.vector.memset(wz[:], 0.0)
                nc.sync.dma_start(warm_in[:], wz[:])
                nc.gpsimd.collective_compute(
                    "AllGather", mybir.AluOpType.bypass,
                    replica_groups=[list(range(CORES))],
                    ins=[warm_in.opt()], outs=[warm_out.opt()])

            # ================= phase 1: feature MLP (paired tiles) ======
            with tc.tile_pool(name="ps1", bufs=2, space="PSUM") as ps1:
                ta = sa1 // P
                for t0 in range(0, NT_MLP, 2):
                    pair = min(2, NT_MLP - t0)
                    tt = tts.tile([P, TC, 2, P], bf, tag="tt")
                    for h_ in range(pair):
                        toff = layout["textT"][0] + (t0 + h_) * P * TC * P
                        eng = nc.sync if h_ == 0 else nc.scalar
                        eng.dma_start(
                            tt[:, :, h_, :],
                            p_blob[0:1, toff:toff + P * TC * P]
                            .rearrange("o (p c n) -> (o p) c n", p=P, c=TC))
                    np_ = pair * P
                    pvT = ps1.tile([P, 2, P], f32, tag="pvT", space="PSUM")
                    nc.tensor.matmul(out=pvT[:, 0:pair, :], lhsT=fc1w[:],
                                     rhs=valT[:, t0 * P:t0 * P + np_],
                                     start=True, stop=True)
                    vT = sb.tile([P, 2, P], bf, tag="vT")
                    nc.vector.tensor_copy(out=vT[:, 0:pair, :],
                                          in_=pvT[:, 0:pair, :])
                    ptT = ps1.tile([P, 2, P], f32, tag="ptT", space="PSUM")
                    for c in range(TC):
                        nc.tensor.matmul(out=ptT[:, 0:pair, :],
                                         lhsT=fc2w[:, c * P:(c + 1) * P],
                                         rhs=tt[:, c, 0:pair, :],
                                         start=(c == 0), stop=(c == TC - 1))
                    tT = sb.tile([P, 2, P], bf, tag="tT")
                    nc.vector.tensor_copy(out=tT[:, 0:pair, :],
                                          in_=ptT[:, 0:pair, :])
                    ph = ps1.tile([P, 2, P], f32, tag="ph", space="PSUM")
                    for h_ in range(pair):
                        nc.tensor.matmul(out=ph[:, h_, :], lhsT=vT[:, h_, :],
                                         rhs=rwv[:], start=True, stop=False)
                        nc.tensor.matmul(out=ph[:, h_, :], lhsT=tT[:, h_, :],
                                         rhs=rwt[:], start=False, stop=False)
                        nc.tensor.matmul(out=ph[:, h_, :], lhsT=ones1[:],
                                         rhs=beff[:], start=False, stop=True)
                    for h_ in range(pair):
                        t = t0 + h_
                        if t < NT1:
                            hdst = hall1[:, t, :]
                        else:
                            hsb = sb.tile([P, P], bf, tag="hsb")
                            hdst = hsb[:]
                        nc.scalar.activation(
                            out=hdst, in_=ph[:, h_, :],
                            func=mybir.ActivationFunctionType.Lrelu,
                            alpha=0.01)
                        nc.sync.dma_start(
                            h_shard[t * P:(t + 1) * P, :], hdst)
                        if t == ta - 1 and "coll" not in ab:
                            nc.gpsimd.collective_compute(
                                "AllGather", mybir.AluOpType.bypass,
                                replica_groups=[list(range(CORES))],
                                ins=[h_shard[0:sa1, :]],
                                outs=[h_fullA.opt()])

            if "coll" not in ab:
                nc.gpsimd.collective_compute(
                    "AllGather", mybir.AluOpType.bypass,
                    replica_groups=[list(range(CORES))],
                    ins=[h_shard[sa1:NSP, :]], outs=[h_fullB.opt()])
            else:
                nc.sync.dma_start(h_fullA[0:sa1, :], h_shard[0:sa1, :])
                nc.sync.dma_start(h_fullB[0:sb1, :], h_shard[sa1:NSP, :])

            selfA = resident("selfA")
            ww1 = resident("ww1")
            b1 = resident("b1")
            ww2 = resident("ww2")
            b2 = resident("b2")
            fc3w = resident("fc3w")
            fc3b = resident("fc3b")
            idxsb = {
                1: [resident("idx1s0", i16), resident("idx1s1", i16)],
                2: [resident("idx2s0", i16), resident("idx2s1", i16)],
            }

            # ================= RGCN layers =================
            def rgcn_layer(lp, src_tabs, hall, ww, bb, layer, out_shards,
                           aseg_name, li, after_tile=None):
                emitted = {0: -1, 1: -1}
                aemitted = {0: -1, 1: -1}
                ebufs = {0: {}, 1: {}}
                abufs = {0: {}, 1: {}}
                pools = {0: elo, 1: ehi}
                nstream = {s: lp.ns[s] for s in range(2)}
                secoff = [0, lp.ns[0]]
                aseg = seg(aseg_name)

                def emit_chunk(s, ci):
                    s0 = ci * CHMAX
                    ns = min(CHMAX, nstream[s] - s0)
                    et = pools[s].tile([P, CHMAX, FEAT], bf, tag=f"e{s}")
                    if "gather" in ab:
                        nc.vector.memset(et[:, 0:1, 0:2], 0.0)
                        ebufs[s][ci] = (et, s0, ns)
                        ebufs[s].pop(ci - 15, None)
                        return
                    src_ap = src_tabs[s]
                    qrr["n"] += 1
                    nc.gpsimd.dma_gather(
                        out_ap=et[:, 0:ns, :],
                        in_ap=src_ap,
                        idxs_ap=idxsb[li][s][:, s0 * 8:(s0 + ns) * 8],
                        num_idxs=ns * P,
                        num_idxs_reg=ns * P,
                        elem_size=FEAT,
                        queue_num=qrr["n"] % 4)
                    ebufs[s][ci] = (et, s0, ns)
                    ebufs[s].pop(ci - 15, None)

                def emit_abatch(s, ai):
                    s0 = ai * ABATCH
                    ns = min(ABATCH, nstream[s] - s0)
                    at = abp.tile([P, ABATCH, W], bf, tag=f"a{s}")
                    if "astream" in ab:
                        nc.vector.memset(at[:, 0:1, 0:2], 0.0)
                    else:
                        g0 = s0 + secoff[s]
                        nc.scalar.dma_start(
                            at[:, 0:ns, :],
                            aseg[:, g0 * W:(g0 + ns) * W])
                    abufs[s][ai] = (at, s0, ns)
                    abufs[s].pop(ai - 2, None)

                with tc.tile_pool(name=f"psl{li}", bufs=2, space="PSUM") as psl:
                    nt = lp.nt
                    for t in range(nt):
                        pS = psl.tile([P, RSLOT * P], f32, tag="pS",
                                      space="PSUM")
                        # self/root one-hot: also initializes all 512 cols
                        nc.tensor.matmul(out=pS[:], lhsT=hall[:, t, :],
                                         rhs=selfA[:], start=True, stop=False,
                                         skip_group_check=True)
                        for s in (0, 1):
                            a, b = lp.tile_slot_range[s][t]
                            for j in range(a, b):
                                ci = j // CHMAX
                                ai = j // ABATCH
                                if ci > emitted[s]:
                                    emit_chunk(s, ci)
                                    emitted[s] = ci
                                if ai > aemitted[s]:
                                    emit_abatch(s, ai)
                                    aemitted[s] = ai
                                et, es0, _ = ebufs[s][ci]
                                at, as0, _ = abufs[s][ai]
                                bj = int(lp.slot_base[s][j])
                                if "slotmm" in ab:
                                    continue
                                nc.tensor.matmul(
                                    out=pS[:, bj:bj + W],
                                    lhsT=et[:, j - es0, :],
                                    rhs=at[:, j - as0, :],
                                    start=False, stop=False,
                                    skip_group_check=True)
                        sS = sb.tile([P, RSLOT * P], bf, tag="sS")
                        nc.scalar.activation(
                            out=sS[:], in_=pS[:],
                            func=mybir.ActivationFunctionType.Copy)
                        if layer == 1:
                            pO = psl.tile([P, FEAT], f32, tag="pO",
                                          space="PSUM")
                            for r in range(RSLOT):
                                nc.tensor.matmul(
                                    out=pO[:], lhsT=sS[:, r::RSLOT],
                                    rhs=ww[:, r * FEAT:(r + 1) * FEAT],
                                    start=(r == 0), stop=False)
                            nc.tensor.matmul(out=pO[:], lhsT=ones1[:],
                                             rhs=bb[:], start=False, stop=True)
                            if t < NT2:
                                hodst = hall2[:, t, :]
                            else:
                                hot = sb.tile([P, P], bf, tag="ho")
                                hodst = hot[:]
                            nc.vector.tensor_copy(out=hodst, in_=pO[:])
                            nc.sync.dma_start(
                                out_shards[t * P:(t + 1) * P, :], hodst)
                        else:
                            pO = psl.tile([P, P], f32, tag="pO", space="PSUM")
                            for r in range(RSLOT):
                                nc.tensor.matmul(
                                    out=pO[:],
                                    lhsT=ww[:, r * FEAT:(r + 1) * FEAT],
                                    rhs=sS[:, r::RSLOT],
                                    start=(r == 0), stop=False)
                            nc.tensor.matmul(out=pO[:], lhsT=bb[:],
                                             rhs=ones1[:], start=False,
                                             stop=True)
                            h2T = sb.tile([P, P], bf, tag="h2T")
                            nc.vector.tensor_copy(out=h2T[:], in_=pO[:])
                            pL = psl.tile([CLASSES, P], f32, tag="pL",
                                          space="PSUM")
                            nc.tensor.matmul(out=pL[:], lhsT=fc3w[:],
                                             rhs=h2T[:], start=True,
                                             stop=False)
                            nc.tensor.matmul(out=pL[:], lhsT=fc3b[:],
                                             rhs=ones1[:], start=False,
                                             stop=True)
                            lg = sb.tile([CLASSES, P], f32, tag="lg")
                            nc.vector.tensor_copy(out=lg[:], in_=pL[:])
                            nc.sync.dma_start(p_logT[:, t * P:(t + 1) * P],
                                              lg[:])
                        if after_tile is not None:
                            after_tile(t)

            rgcn_layer(pl.L1, (h_fullA[:, :], h_fullB[:, :]),
                       hall1, ww1, b1, 1, h1_shard, "a1", 1)
            if "coll" not in ab:
                nc.gpsimd.collective_compute(
                    "AllGather", mybir.AluOpType.bypass,
                    replica_groups=[list(range(CORES))],
                    ins=[h1_shard.opt()], outs=[h1_full.opt()])
            else:
                nc.sync.dma_start(h1_full[0:NT1 * P, :], h1_shard[:])
            lo2 = min(NROWS2, 32768)
            hi2 = pl.hibase2
            rgcn_layer(pl.L2,
                       (h1_full[0:lo2, :], h1_full[hi2:NROWS2, :]),
                       hall2, ww2, b2, 2, None, "a2", 2)

    nc.compile()
    return nc


# ============================ host packing =============================

def pack_inputs(pl, inputs):
    f32 = np.float32
    sn = pl.sigma["shard_nodes"]  # [CORES, NSP], -1 pad
    vf = np.asarray(inputs["value_feature"], f32)
    tf = np.asarray(inputs["text_feature"], f32)

    def shard_textT(c):
        x = np.zeros((NSP, TEXT), f32)
        valid = sn[c] >= 0
        x[valid] = tf[sn[c][valid]]
        y = x.reshape(NT_MLP, P, TC, P).transpose(0, 3, 2, 1)
        return np.ascontiguousarray(y.reshape(NT_MLP, P, TC * P).astype(BF16))

    def shard_valT(c):
        x = np.zeros((NSP, VAL), f32)
        valid = sn[c] >= 0
        x[valid] = vf[sn[c][valid]]
        return np.ascontiguousarray(x.T.astype(BF16))

    fc1w = np.asarray(inputs["fc1_w"], f32)
    fc2w = np.asarray(inputs["fc2_w"], f32)
    relw = np.asarray(inputs["relu_w"], f32)
    beff = (np.concatenate([np.asarray(inputs["fc1_b"], f32),
                            np.asarray(inputs["fc2_b"], f32)]) @ relw
            + np.asarray(inputs["relu_b"], f32))
    fc2w_t = np.ascontiguousarray(
        fc2w.reshape(TC, P, FEAT).transpose(1, 0, 2)
        .reshape(P, TC * FEAT).astype(BF16))

    def stack_w(wrel, wroot):
        w = np.concatenate([np.asarray(wrel, f32),
                            np.asarray(wroot, f32)[None]], 0)
        return np.ascontiguousarray(
            w.transpose(1, 0, 2).reshape(P, RSLOT * FEAT).astype(BF16))

    selfA = np.zeros((P, RSLOT * P), f32)
    selfA[np.arange(P), np.arange(P) * RSLOT + (RSLOT - 1)] = 1.0

    layout, blob_n = blob_layout(pl)
    shared = dict(
        fc1w=fc1w.astype(BF16), fc2w=fc2w_t,
        rwv=np.ascontiguousarray(relw[:FEAT].astype(BF16)),
        rwt=np.ascontiguousarray(relw[FEAT:].astype(BF16)),
        beff=beff[None].astype(BF16),
        ww1=stack_w(inputs["rgcn1_wrel"], inputs["rgcn1_wroot"]),
        b1=np.asarray(inputs["rgcn1_b"], f32)[None].astype(BF16),
        ww2=stack_w(inputs["rgcn2_wrel"], inputs["rgcn2_wroot"]),
        b2=np.asarray(inputs["rgcn2_b"], f32)[None].astype(BF16),
        fc3w=np.asarray(inputs["fc3_w"], f32).astype(BF16),
        fc3b=np.asarray(inputs["fc3_b"], f32)[None].astype(BF16),
        ones1=np.ones((1, P), f32).astype(BF16),
        selfA=selfA.astype(BF16),
    )

    def idxseg(arr):
        return (wrap16(arr.reshape(-1)) if arr.size
                else np.zeros((P, 8), np.int16)).view(BF16)

    in_maps = []
    for c in range(CORES):
        vals = dict(shared)
        vals["textT"] = shard_textT(c)
        vals["valT"] = shard_valT(c)
        vals["a1"] = pl.L1.amat[c] if pl.L1.nslot else np.zeros((P, W), BF16)
        vals["a2"] = pl.L2.amat[c] if pl.L2.nslot else np.zeros((P, W), BF16)
        for s in range(2):
            vals[f"idx1s{s}"] = idxseg(pl.L1.idx[c][s])
            vals[f"idx2s{s}"] = idxseg(pl.L2.idx[c][s])
        blob = np.zeros((1, blob_n), BF16)
        for name, (off, n, shape) in layout.items():
            a = vals[name]
            assert a.size == n, (name, a.shape, shape)
            blob[0, off:off + n] = a.reshape(-1)
        in_maps.append({"blob": blob})
    return in_maps


# ============================ entry point =============================

_cache = {}


def kernel(**inputs):
    ei = np.asarray(inputs["edge_index"], np.int64)
    et = np.asarray(inputs["edge_type"], np.int64)
    idx = np.asarray(inputs["idx"], np.int64)

    key = hash((ei.tobytes(), et.tobytes(), idx.tobytes()))
    if key not in _cache:
        pl = make_plan(ei, et, idx)
        nc = build_bass(pl)
        _cache[key] = (pl, nc)
    pl, nc = _cache[key]

    in_maps = pack_inputs(pl, inputs)
    res = run_bass_kernel_spmd(nc, in_maps, list(range(CORES)))
    return assemble(pl, res, idx)


def assemble(pl, res, idx):
    sn = pl.sigma["shard_nodes"]
    logits = np.zeros((N_NODES, CLASSES), np.float32)
    for c in range(CORES):
        lt = res.results[c]["logitsT"]  # [2, NT2*P]
        nodes = sn[c][:pl.nt2 * P]
        vv = nodes >= 0
        logits[nodes[vv]] = lt[:, :len(nodes)][:, vv].T
    return logits[np.asarray(idx, np.int64)].astype(np.float32)

